# revision 17
# baseline (speedup 1.0000x reference)
"""Trainium2 Bass kernel for nn_MultiHeadAttention_755914244781.

Per-position MHA variant: scores contract over DK within each (b,s) position,
scores/probs are [B,S,H,H]; returns (out [B,S,H,D], probs).

Sharding: data-parallel over flattened (B*S)=4096 positions, 512 per core,
weights replicated. Each core runs an identical Bass program on its slice.
"""
import os
import numpy as np

B, S, D, H, DK = 2, 2048, 1024, 16, 64
NPOS = B * S                  # 4096
NCORES = 8
POS = NPOS // NCORES          # 512 positions per core
T = 256                       # positions per half
NHALF = POS // T              # 2
NG = T // 8                   # 32 groups of 8 positions per half
KC = D // 128                 # 8 contraction chunks
HP = H // 2                   # 8 head pairs

_cached = {}


def _build_nc():
    import concourse.bass as bass
    import concourse.tile as tile
    from concourse import bacc, mybir
    from concourse.masks import make_identity
    from contextlib import ExitStack

    f32 = mybir.dt.float32
    f32r = mybir.dt.float32r
    bf16 = mybir.dt.bfloat16
    EXP = mybir.ActivationFunctionType.Exp
    COPY = mybir.ActivationFunctionType.Copy

    nc = bacc.Bacc(
        "TRN2", target_bir_lowering=False, debug=False,
        enable_asserts=False, num_devices=NCORES,
    )

    # DRAM tensors (per-core slices; weights replicated)
    dq = nc.dram_tensor("q", [POS, D], f32, kind="ExternalInput").ap()
    dk = nc.dram_tensor("k", [POS, D], f32, kind="ExternalInput").ap()
    dv = nc.dram_tensor("v", [POS, D], f32, kind="ExternalInput").ap()
    dWq = nc.dram_tensor("Wq", [D, D], f32, kind="ExternalInput").ap()
    dWk = nc.dram_tensor("Wk", [D, D], f32, kind="ExternalInput").ap()
    dWv = nc.dram_tensor("Wv", [D, D], f32, kind="ExternalInput").ap()
    dWo = nc.dram_tensor("Wo", [DK, D], f32, kind="ExternalInput").ap()
    dbq = nc.dram_tensor("bq", [D], f32, kind="ExternalInput").ap()
    dbk = nc.dram_tensor("bk", [D], f32, kind="ExternalInput").ap()
    dbv = nc.dram_tensor("bv", [D], f32, kind="ExternalInput").ap()
    dbo = nc.dram_tensor("bo", [D], f32, kind="ExternalInput").ap()
    dout = nc.dram_tensor("out", [POS * H, D], f32, kind="ExternalOutput").ap()
    dprobs = nc.dram_tensor("probs", [POS * H, H], f32, kind="ExternalOutput").ap()

    with ExitStack() as ctx:
        tc = ctx.enter_context(tile.TileContext(nc))

        wpool = ctx.enter_context(tc.tile_pool(name="wpool", bufs=1))
        bigp = ctx.enter_context(tc.tile_pool(name="bigp", bufs=6))
        projp = ctx.enter_context(tc.tile_pool(name="projp", bufs=1))
        smallp = ctx.enter_context(tc.tile_pool(name="smallp", bufs=4))
        persist = ctx.enter_context(tc.tile_pool(name="persist", bufs=1))
        outp = ctx.enter_context(tc.tile_pool(name="outp", bufs=3))
        psA = ctx.enter_context(tc.tile_pool(name="psA", bufs=2, space="PSUM"))
        psB = ctx.enter_context(tc.tile_pool(name="psB", bufs=3, space="PSUM"))
        psO = ctx.enter_context(tc.tile_pool(name="psO", bufs=3, space="PSUM"))

        # ---- constants ----
        ident = wpool.tile([128, 128], f32)
        make_identity(nc, ident)

        w_sb = {}
        for nm, dW in (("q", dWq), ("k", dWk), ("v", dWv)):
            wt = wpool.tile([128, KC * D], f32r, tag=f"W{nm}")
            nc.gpsimd.dma_start(
                out=wt.rearrange("r (kc j) -> r kc j", kc=KC),
                in_=dW.rearrange("(kc r) j -> r kc j", r=128),
            )
            w_sb[nm] = wt

        wo_aug = wpool.tile([DK + 1, D], f32r)
        nc.gpsimd.dma_start(out=wo_aug[0:DK, :], in_=dWo)
        nc.gpsimd.dma_start(out=wo_aug[DK:DK + 1, :],
                          in_=dbo.rearrange("(o j) -> o j", o=1))

        bq_sb = wpool.tile([128, HP], f32)
        nc.sync.dma_start(out=bq_sb, in_=dbq.rearrange("(hp r) -> r hp", r=128))
        bk_sb = wpool.tile([128, HP], f32)
        nc.sync.dma_start(out=bk_sb, in_=dbk.rearrange("(hp r) -> r hp", r=128))
        bv_sb = wpool.tile([DK, H], f32)
        nc.sync.dma_start(out=bv_sb, in_=dbv.rearrange("(t d) -> d t", d=DK))

        # E_bd double buffers: pair-block-diagonal expT, off-blocks stay zero
        e_bds = []
        for i in range(2):
            e = persist.tile([128, 128], f32, tag=f"ebd{i}", name=f"ebd{i}")
            nc.vector.memset(e, 0.0)
            e_bds.append(e)
        # K=1 constant rows that pre-bias the scoresT psum with -600 on the
        # within-pair off-diagonal 16x16 blocks (exp then sends them to ~0).
        rowU_A = persist.tile([1, 128], bf16, tag="rowUA")
        rowU_B = persist.tile([1, 128], bf16, tag="rowUB")
        rowV_A = persist.tile([1, 32], bf16, tag="rowVA")
        rowV_B = persist.tile([1, 32], bf16, tag="rowVB")
        nc.vector.memset(rowU_A, 0.0)
        nc.vector.memset(rowU_B, 0.0)
        nc.vector.memset(rowV_A, 0.0)
        nc.vector.memset(rowV_B, 0.0)
        for blk in range(4):
            lo, hi = blk * 32, blk * 32 + 16
            nc.vector.memset(rowU_A[0:1, lo:hi], 1.0)          # parity(r)==0
            nc.vector.memset(rowU_B[0:1, hi:blk * 32 + 32], 1.0)  # parity(r)==1
        nc.vector.memset(rowV_A[0:1, 16:32], -600.0)
        nc.vector.memset(rowV_B[0:1, 0:16], -600.0)

        for half in range(NHALF):
            # ---- stage A: load + transpose inputs ----
            qT = {}
            for nm, dx in (("q", dq), ("k", dk), ("v", dv)):
                xn = bigp.tile([128, 2 * D], f32, tag="big")
                nc.sync.dma_start(
                    out=xn.rearrange("r (s j) -> r s j", s=2),
                    in_=dx.rearrange("(hf s r) j -> hf r s j", s=2, r=128)[half],
                )
                xT = bigp.tile([128, KC * T], f32r, tag="big")
                for kc in range(KC):
                    for s in range(2):
                        pt = psA.tile([128, 128], f32, tag="psA")
                        nc.tensor.transpose(
                            pt,
                            xn[:, s * D + kc * 128: s * D + (kc + 1) * 128],
                            ident,
                        )
                        nc.vector.tensor_copy(
                            xT[:, kc * T + s * 128: kc * T + (s + 1) * 128], pt)
                qT[nm] = xT

            # ---- stage B: projections ----
            # q, k: head-pair packed [128=(parity,d), HP*T]
            qd2 = {}
            for nm, bias in (("q", bq_sb), ("k", bk_sb)):
                acc = projp.tile([128, HP * T], f32, tag=f"{nm}d2")
                for hp in range(HP):
                    ps = psA.tile([128, T], f32, tag="psA")
                    for kc in range(KC):
                        nc.tensor.matmul(
                            ps,
                            w_sb[nm][:, kc * D + hp * 128: kc * D + (hp + 1) * 128],
                            qT[nm][:, kc * T:(kc + 1) * T],
                            start=(kc == 0), stop=(kc == KC - 1),
                        )
                    nc.vector.tensor_scalar_add(
                        acc[:, hp * T:(hp + 1) * T], ps, bias[:, hp:hp + 1])
                qd2[nm] = acc
            # v: [64 d, p*16+t] (position-major cols so Vstack transpose input
            # is a contiguous [64,128] slice — matmul APs allow only 1 free dim)
            vd = projp.tile([DK, H * T], f32, tag="vd")
            vd_pm = vd.rearrange("d (p t) -> d t p", t=H)
            for t in range(H):
                ps = psA.tile([DK, T], f32, tag="psA")
                for kc in range(KC):
                    nc.tensor.matmul(
                        ps,
                        w_sb["v"][:, kc * D + t * DK: kc * D + (t + 1) * DK],
                        qT["v"][:, kc * T:(kc + 1) * T],
                        start=(kc == 0), stop=(kc == KC - 1),
                    )
                nc.vector.tensor_scalar_add(vd_pm[:, t, :], ps, bv_sb[:, t:t + 1])

            # ---- stage C: assemble block-diag L / stacked R operands (bf16) ----
            # Qd2 col = c*T + 2i + p2 ; view (r, p2, i, c)
            qsrc = qd2["q"].rearrange("r (c i p2) -> r p2 i c", p2=2, c=HP)
            ksrc = qd2["k"].rearrange("r (c i p2) -> r p2 i c", p2=2, c=HP)

            L_q = bigp.tile([128, 32 * (T // 2)], bf16, tag="big")
            R_k = bigp.tile([128, 16 * (T // 2)], bf16, tag="big")
            L_k = bigp.tile([128, 32 * (T // 2)], bf16, tag="big")
            nc.gpsimd.memset(L_q, 0.0)
            nc.gpsimd.memset(L_k, 0.0)

            # L_q col = 32i + 16*p2 + 2c + par   view (r, i, p2, par, c)
            ld = L_q.rearrange("r (i p2 c par) -> r i p2 par c", p2=2, c=HP, par=2)
            nc.gpsimd.tensor_copy(ld[0:64, :, 0, 0, :], qsrc[0:64, 0, :, :])
            nc.gpsimd.tensor_copy(ld[0:64, :, 0, 1, :], qsrc[64:128, 0, :, :])
            nc.gpsimd.tensor_copy(ld[64:128, :, 1, 0, :], qsrc[0:64, 1, :, :])
            nc.gpsimd.tensor_copy(ld[64:128, :, 1, 1, :], qsrc[64:128, 1, :, :])
            # R_k col = 16i + 2c + par (t index), rows (p2*64+d)
            rd = R_k.rearrange("r (i c par) -> r i par c", c=HP, par=2)
            nc.gpsimd.tensor_copy(rd[0:64, :, 0, :], ksrc[0:64, 0, :, :])
            nc.gpsimd.tensor_copy(rd[0:64, :, 1, :], ksrc[64:128, 0, :, :])
            nc.gpsimd.tensor_copy(rd[64:128, :, 0, :], ksrc[0:64, 1, :, :])
            nc.gpsimd.tensor_copy(rd[64:128, :, 1, :], ksrc[64:128, 1, :, :])
            # L_k: same as L_q but from k
            ld = L_k.rearrange("r (i p2 c par) -> r i p2 par c", p2=2, c=HP, par=2)
            nc.gpsimd.tensor_copy(ld[0:64, :, 0, 0, :], ksrc[0:64, 0, :, :])
            nc.gpsimd.tensor_copy(ld[0:64, :, 0, 1, :], ksrc[64:128, 0, :, :])
            nc.gpsimd.tensor_copy(ld[64:128, :, 1, 0, :], ksrc[0:64, 1, :, :])
            nc.gpsimd.tensor_copy(ld[64:128, :, 1, 1, :], ksrc[64:128, 1, :, :])

            # ---- stages D/E per 8-position group ----
            for g in range(NG):
                if g % 4 == 0:
                    probs_st = smallp.tile([128, 4, 16], f32, tag="probsst")

                psH = psB.tile([128, 16], f32, tag="psB")
                psT = psB.tile([128, 32], f32, tag="psB")
                for j in range(4):
                    i = g * 4 + j
                    nc.tensor.matmul(psH[32 * j:32 * j + 32, :],
                                     L_q[:, 32 * i:32 * i + 32],
                                     R_k[:, 16 * i:16 * i + 16],
                                     start=True, stop=True,
                                     tile_position=(0, 32 * j))
                # scoresT as [32,32] pair blocks with exact-zero off-diagonals;
                # pre-bias those off-diagonal 16-blocks to -600 so exp -> ~0.
                nc.tensor.matmul(psT, rowU_A, rowV_A, start=True, stop=False,
                                 skip_group_check=True)
                nc.tensor.matmul(psT, rowU_B, rowV_B, start=False, stop=False,
                                 skip_group_check=True)
                for j in range(4):
                    i = g * 4 + j
                    nc.tensor.matmul(psT[32 * j:32 * j + 32, :],
                                     L_k[:, 32 * i:32 * i + 32],
                                     L_q[:, 32 * i:32 * i + 32],
                                     start=False, stop=(j == 3),
                                     tile_position=(0, 32 * j),
                                     skip_group_check=True)

                # softmax (no max-subtraction; scale=1/sqrt(DK) inside exp)
                probs_e = smallp.tile([128, 16], f32, tag="probse")
                sums = smallp.tile([128, 1], f32, tag="sums")
                nc.scalar.activation(out=probs_e, in_=psH, func=EXP, scale=0.125,
                                     accum_out=sums)

                recip = smallp.tile([128, 1], f32, tag="recip")
                nc.vector.reciprocal(recip, sums)
                nc.vector.tensor_scalar_mul(probs_st[:, g % 4, :], probs_e, recip)

                # unnormalized expT straight into the 32-aligned pair blocks
                e_bd = e_bds[g % 2]
                for a in range(4):
                    nc.scalar.activation(
                        out=e_bd[32 * a:32 * a + 32, 32 * a:32 * a + 32],
                        in_=psT[32 * a:32 * a + 32, :], func=EXP, scale=0.125)

                # Vstack via PE transpose of Vd slice [64, (pp,t)]
                # Vstack with an appended ones column: the ctx matmul's
                # row 64 then computes the softmax denominators for free.
                vst_ps = psB.tile([128, DK], f32, tag="psB")
                nc.tensor.transpose(vst_ps,
                                    vd[:, g * 128:(g + 1) * 128],
                                    ident[0:DK, 0:DK])
                vst = smallp.tile([128, DK + 1], f32, tag="vst")
                nc.vector.tensor_copy(vst[:, 0:DK], vst_ps)
                nc.vector.memset(vst[:, DK:DK + 1], 1.0)

                # ctx_u = Vstack_aug.T @ E_bd -> [65 = 64 d + denom, 128 (p,h)]
                # (plain fp32: f32r is 4 cyc/row below N=256 anyway)
                ctxu = psB.tile([DK + 1, 128], f32, tag="psB")
                nc.tensor.matmul(ctxu, vst, e_bd, start=True, stop=True)

                slab = smallp.tile([DK + 1, 128], f32r, tag="slab")
                nc.vector.tensor_copy(slab, ctxu)

                out_sb = outp.tile([128, D], f32, tag="outsb")
                for jh in range(2):
                    po = psO.tile([128, 512], f32, tag="psO")
                    nc.tensor.matmul(
                        po, slab, wo_aug[:, jh * 512:(jh + 1) * 512],
                        start=True, stop=True)
                    nc.scalar.activation(
                        out=out_sb[:, jh * 512:(jh + 1) * 512], in_=po,
                        func=COPY, scale=recip)
                base = (half * T + g * 8) * H
                nc.sync.dma_start(out=dout[base:base + 128, :], in_=out_sb)

                if g % 4 == 3:
                    gb = half * (NG // 4) + g // 4
                    nc.sync.dma_start(
                        out=dprobs.rearrange("(gb g r) t -> gb r g t", g=4, r=128)[gb],
                        in_=probs_st)

    nc.compile()
    return nc


def _get_nc():
    if "nc" not in _cached:
        _cached["nc"] = _build_nc()
    return _cached["nc"]


def run_sharded(inputs, trace=False, **spmd_kwargs):
    from concourse.bass_utils import run_bass_kernel_spmd

    nc = _get_nc()
    f = lambda x: np.ascontiguousarray(np.asarray(x, dtype=np.float32))
    q = f(inputs["q"]).reshape(NPOS, D)
    k = f(inputs["k"]).reshape(NPOS, D)
    v = f(inputs["v"]).reshape(NPOS, D)
    shared = {nm: f(inputs[nm]) for nm in
              ("Wq", "Wk", "Wv", "Wo", "bq", "bk", "bv", "bo")}
    in_maps = []
    for c in range(NCORES):
        sl = slice(c * POS, (c + 1) * POS)
        in_maps.append(dict(q=q[sl], k=k[sl], v=v[sl], **shared))

    res = run_bass_kernel_spmd(nc, in_maps, core_ids=list(range(NCORES)),
                               trace=trace, **spmd_kwargs)
    out = np.empty((NPOS * H, D), np.float32)
    probs = np.empty((NPOS * H, H), np.float32)
    for c in range(NCORES):
        out[c * POS * H:(c + 1) * POS * H] = res.results[c]["out"]
        probs[c * POS * H:(c + 1) * POS * H] = res.results[c]["probs"]
    return out.reshape(B, S, H, D), probs.reshape(B, S, H, H), res


def kernel(**inputs):
    out, probs, _ = run_sharded(inputs)
    return (out, probs)
